# revision 43
# baseline (speedup 1.0000x reference)
"""Trainium2 Bass kernel for nn_BCIM_45861660787130 (pooling / box-filter sim).

Math per sample (C=128 channels, 32x32 spatial = S=1024 pixels):
  unit = p / ||p||_C
  wmean = 3x3 zero-padded box mean of unit (per channel)
  sim = <unit, wmean>_C          # per pixel
  out = p * sim, then channel deinterleave c=(f*2+e) -> [e*S + s, f]

Layout strategy (per core, data-parallel over batch):
  - DMA in sample as [c=128 partitions, s=1024 free] (contiguous).
  - PE transposes 128x128 chunks -> pT [s=128, c=128] in PSUM (NS samples/group
    share one PSUM tile [128, NS*128]).
  - ACT Square with accum_out -> ss[128,1] per sample (fused square+reduce),
    ACT sqrt -> nrm, DVE reciprocal -> rinv.
  - ACT Copy(scale=rinv) evacuates PSUM -> u (normalized, SBUF, bf16).
  - Box filter = block-tridiagonal matmuls on PE (bf16, N=512):
      box_k = Bd^T u_k + Bp^T u_{k-1} + Bn^T u_{k+1}  (PSUM accumulate)
    where Bd/Bp/Bn are constant 128x128 banded matrices (entries 1/9).
  - DVE tensor_tensor_reduce per sample: z[s] = sum_c u*box.
  - out = p * sim = u*nrm*z = pT * z: ACT/DVE scalar-mult straight from the
    pT PSUM tile (scale=z per partition), written deinterleaved [k,e,f].
  - DMA out per sample: [128, 8, 2, 64] -> DRAM [2, 8, 128, 64].
"""

import os
import sys

sys.path.insert(0, "/opt/trn_rl_repo")

import numpy as np

from concourse import bacc, bass, mybir, tile
from concourse.bass_utils import run_bass_kernel_spmd

F32 = mybir.dt.float32
BF16 = mybir.dt.bfloat16
AF = mybir.ActivationFunctionType
ALU = mybir.AluOpType
AX = mybir.AxisListType

B_PER_CORE = 32  # samples per core
NS = 4  # samples per group (matmul N = NS*128 = 512)
NG = B_PER_CORE // NS
NK = 8  # s-chunks per sample (1024 / 128)
C = 128
S = 1024

# engine split knobs (ACT vs DVE)
OA = int(os.environ.get("OA", 4))  # out-scales on ACT (rest DVE)
SQ_ON_ACT = os.environ.get("SQ_ON_ACT", "0") == "1"
U_BF16 = os.environ.get("U_BF16", "1") == "1"
UDT = BF16 if U_BF16 else mybir.dt.float32r  # dtype of u tiles (box inputs)
OUT_FROM_PT = os.environ.get("OUT_FROM_PT", "1") == "1"
P_F32R = os.environ.get("P_F32R", "1") == "1"  # f32r input -> 1.5cyc transposes
U_BIG = os.environ.get("U_BIG", "1") == "1"  # u-scale as one DVE bcast op
Z_STT = os.environ.get("Z_STT", "0") == "1"  # z via scalar_tensor_tensor accum
RED_BF16 = os.environ.get("RED_BF16", "0") == "1"  # bf16 sq/wscr (no 2x on HW)


def _consts():
    t32 = (np.abs(np.subtract.outer(np.arange(32), np.arange(32))) <= 1).astype(
        np.float32
    )
    a4 = (np.abs(np.subtract.outer(np.arange(4), np.arange(4))) <= 1).astype(
        np.float32
    )
    e30 = np.zeros((4, 4), np.float32)
    e30[3, 0] = 1.0
    e03 = np.zeros((4, 4), np.float32)
    e03[0, 3] = 1.0
    bd = np.kron(a4, t32) / 9.0
    bp = np.kron(e30, t32) / 9.0  # from chunk k-1
    bn = np.kron(e03, t32) / 9.0  # from chunk k+1
    ident = np.eye(128, dtype=np.float32)
    wbox = np.stack([bd, bp, bn]).astype(np.float32)
    return ident, wbox


def build_nc():
    nc = bacc.Bacc()
    PDT = mybir.dt.float32r if P_F32R else F32
    p_d = nc.declare_dram_parameter("p", [B_PER_CORE, C, S], PDT, isOutput=False)
    out_d = nc.declare_dram_parameter(
        "out", [B_PER_CORE, 2, NK, 128, 64], F32, isOutput=True
    )
    ident_d = nc.declare_dram_parameter("ident", [128, 128], PDT, isOutput=False)
    WDT = F32 if U_BF16 else mybir.dt.float32r
    wbox_d = nc.declare_dram_parameter("wbox", [3, 128, 128], WDT, isOutput=False)

    with tile.TileContext(nc) as tc:
        with (
            tc.tile_pool(name="consts", bufs=1) as cpool,
            tc.tile_pool(name="pin", bufs=4 * NS) as pin,
            tc.tile_pool(name="upool", bufs=2 * NK) as upool,
            tc.tile_pool(name="sq", bufs=4) as sqpool,
            tc.tile_pool(name="wscr", bufs=6) as wpool,
            tc.tile_pool(name="outp", bufs=3 * NS) as outpool,
            tc.tile_pool(name="stats", bufs=6 * NK) as stats,
            tc.tile_pool(name="psT", bufs=6, space="PSUM") as psT,
            tc.tile_pool(name="psB", bufs=2, space="PSUM") as psB,
        ):
            ident = cpool.tile([128, 128], PDT, tag="ident")
            wboxf = cpool.tile([128, 3, 128], WDT, tag="wboxf")
            nc.sync.dma_start(ident[:], ident_d[:])
            nc.sync.dma_start(wboxf[:], wbox_d[:].transpose([1, 0, 2]))
            if U_BF16:
                wbox = cpool.tile([128, 3, 128], UDT, tag="wbox")
                nc.scalar.activation(wbox[:], wboxf[:], AF.Copy)
                bd, bp, bn = wbox[:, 0, :], wbox[:, 1, :], wbox[:, 2, :]
            else:
                bd, bp, bn = (wboxf[:, j, :] for j in range(3))

            # startup observers: make PE's vector clock see both const-DMA
            # queue sems so steady-state matmuls never wait on them (matmuls
            # only support a single sync wait in codegen).
            identf = ident[:].bitcast(F32) if P_F32R else ident[:]
            scr1 = psT.tile([128, 1], F32, tag="pT")
            nc.tensor.matmul(
                scr1[:], identf, identf[:, 0:1], start=True, stop=True
            )
            scr2 = psB.tile([128, 1], F32, tag="box")
            scr2_rhs = wboxf[:, 0, 0:1]
            if not U_BF16:
                scr2_rhs = scr2_rhs.bitcast(F32)
            nc.tensor.matmul(
                scr2[:], identf, scr2_rhs, start=True, stop=True
            )

            all_ptiles = []
            for g in range(NG):
                gp = []
                for b in range(NS):
                    pt = pin.tile([C, S], PDT, tag="pt", name=f"pt_{g}_{b}")
                    nc.sync.dma_start(pt[:], p_d[g * NS + b])
                    gp.append(pt)
                all_ptiles.append(gp)
            for g in range(NG):
                ptiles = all_ptiles[g]

                outts = [
                    outpool.tile([128, NK, 2, 64], F32, tag="ot", name=f"ot_{g}_{b}")
                    for b in range(NS)
                ]
                pTs, us, zs, nrms = {}, {}, {}, {}

                # interleaved pipeline: norm(k), box+z(k-BO), out(k-OO) --
                # box consumes u finished BO-1 steps ago (PE never stalls, so
                # transposes are never delayed behind a waiting box matmul)
                # and out-scales consume z finished OO-BO steps ago. The last
                # group shortens the offsets: no transposes follow, so the
                # stall-protection is pointless and only lengthens the drain.
                BO, OO = (3, 4) if g < NG - 1 else (2, 2)
                for kk in range(NK + OO):
                    if kk < NK:
                        k = kk
                        pT = psT.tile([128, NS, 128], PDT, tag="pT")
                        for b in range(NS):
                            nc.tensor.transpose(
                                pT[:, b, :],
                                ptiles[b][:, k * 128 : (k + 1) * 128],
                                ident[:],
                            )
                        pTf = pT[:].bitcast(F32) if P_F32R else pT[:]
                        RDT = BF16 if RED_BF16 else F32
                        ss = stats.tile([128, NS], RDT, tag="ss")
                        sq = sqpool.tile([128, NS, 128], RDT, tag="sq")
                        if SQ_ON_ACT:
                            for b in range(NS):
                                nc.scalar.activation(
                                    sq[:, b, :],
                                    pTf[:, b, :],
                                    AF.Square,
                                    accum_out=ss[:, b : b + 1],
                                )
                        else:
                            nc.scalar.activation(sq[:], pTf[:, :, :], AF.Square)
                            with nc.allow_low_precision("bf16 norm stats ok"):
                                nc.vector.tensor_reduce(
                                    ss[:], sq[:], axis=AX.X, op=ALU.add
                                )
                        nrm = stats.tile([128, NS], F32, tag="nrm")
                        nc.scalar.sqrt(nrm[:], ss[:])
                        rinv = stats.tile([128, NS], F32, tag="rinv")
                        nc.vector.reciprocal(rinv[:], nrm[:])
                        ua = upool.tile([128, NS, 128], UDT, tag="ua")
                        if U_BIG:
                            rv = rinv[:].unsqueeze(-1).broadcast_to(
                                [128, NS, 128]
                            )
                            nc.vector.tensor_tensor(
                                ua[:], pTf[:, :, :], rv, op=ALU.mult
                            )
                        else:
                            for b in range(NS):
                                nc.scalar.activation(
                                    ua[:, b, :],
                                    pTf[:, b, :],
                                    AF.Copy,
                                    scale=rinv[:, b : b + 1],
                                )
                        pTs[k], us[k], nrms[k] = pTf, ua, nrm

                    if kk >= BO and kk - BO < NK:
                        k = kk - BO
                        box = psB.tile([128, NS, 128], F32, tag="box")
                        mms = [(bd, k)]
                        if k > 0:
                            mms.append((bp, k - 1))
                        if k < NK - 1:
                            mms.append((bn, k + 1))
                        for i, (w, j) in enumerate(mms):
                            nc.tensor.matmul(
                                box[:],
                                w,
                                us[j][:],
                                start=(i == 0),
                                stop=(i == len(mms) - 1),
                            )
                        ZDT = BF16 if RED_BF16 else F32
                        z = stats.tile([128, NS], F32, tag="z")
                        wscr = wpool.tile([128, NS, 128], ZDT, tag="w")
                        u_in = us[k][:]
                        if not U_BF16:
                            u_in = u_in.bitcast(F32)
                        # NOTE: tensor_tensor_reduce wedges the device on
                        # this runtime; scalar_tensor_tensor w/ accum works.
                        if Z_STT:
                            for b in range(NS):
                                nc.vector.scalar_tensor_tensor(
                                    out=wscr[:, b, :],
                                    in0=u_in[:, b, :],
                                    scalar=1.0,
                                    in1=box[:, b, :],
                                    op0=ALU.mult,
                                    op1=ALU.mult,
                                    accum_out=z[:, b : b + 1],
                                )
                        else:
                            nc.vector.tensor_tensor(
                                wscr[:], u_in, box[:], op=ALU.mult
                            )
                            with nc.allow_low_precision("bf16 sim ok"):
                                nc.vector.tensor_reduce(
                                    z[:], wscr[:], axis=AX.X, op=ALU.add
                                )
                        zs[k] = z

                    if kk >= OO and kk - OO < NK:
                        k = kk - OO
                        # out = pT * z  (== p * sim), deinterleaved [e, f]
                        z = zs[k]
                        pT = pTs[k]
                        if OUT_FROM_PT:
                            fs = z
                        else:
                            fs = stats.tile([128, NS], F32, tag="fs")
                            nc.vector.tensor_mul(fs[:], z[:], nrms[k][:])
                        for b in range(NS):
                            if OUT_FROM_PT:
                                pv = pT[:, b, :]
                            else:
                                pv = us[k][:, b, :]
                                if not U_BF16:
                                    pv = pv.bitcast(F32)
                            pv = pv.rearrange("p (f e) -> p e f", e=2)
                            if b < OA:
                                nc.scalar.activation(
                                    outts[b][:, k, :, :],
                                    pv,
                                    AF.Copy,
                                    scale=fs[:, b : b + 1],
                                )
                            else:
                                nc.vector.tensor_scalar_mul(
                                    outts[b][:, k, :, :], pv, fs[:, b : b + 1]
                                )

                for b in range(NS):
                    for e in range(2):
                        # halves: chunks 0-3 flush while 4-7 still compute
                        for h in range(2):
                            ks = slice(h * (NK // 2), (h + 1) * (NK // 2))
                            dst = out_d[g * NS + b, e, ks].transpose([1, 0, 2])
                            nc.sync.dma_start(dst, outts[b][:, ks, e, :])

    nc.compile()
    return nc


_CACHE = {}


def _get_nc():
    if "nc" not in _CACHE:
        _CACHE["nc"] = build_nc()
    return _CACHE["nc"]


def kernel(p_vector: np.ndarray) -> np.ndarray:
    p = np.ascontiguousarray(p_vector, dtype=np.float32)
    assert p.shape == (256, 128, 32, 32)
    shards = p.reshape(8, B_PER_CORE, C, S)
    ident, wbox = _consts()
    nc = _get_nc()
    wbox_rep = _to_dev_dtype(wbox)
    in_maps = [
        {"p": shards[i], "ident": ident, "wbox": wbox_rep} for i in range(8)
    ]
    res = run_bass_kernel_spmd(nc, in_maps, core_ids=list(range(8)))
    outs = [r["out"].reshape(B_PER_CORE, 2048, 64) for r in res.results]
    return np.concatenate(outs, axis=0)


def _to_dev_dtype(wbox: np.ndarray) -> np.ndarray:
    return wbox  # wbox ships as fp32; on-chip cast handles UDT


if __name__ == "__main__":
    x = np.random.randn(256, 128, 32, 32).astype(np.float32)
    y = kernel(x)
    print(y.shape, y.dtype)
